# revision 35
# baseline (speedup 1.0000x reference)
"""AdaptiveSpectralRefinement Trainium2 kernel (8 NeuronCores, data-parallel over batch).

v2 design (from v1/307us baseline; see kernel_baseline.py):
  * Spectral stage (rfft->mask->irfft) folded host-side into two 96x96
    matrices; one PE pass per sample emits xl^T and xh^T with d on partitions.
  * Channel matmuls: xl stream quantized to fp8-e4m3 (weights pre-scaled x16)
    and run in DoubleRow perf mode (2 K=256 passes for zl and for the xl half
    of the gate); xh stream stays bf16 (accuracy: the xh band carries 65/96 of
    the energy). Simulated end-to-end rel err 1.16e-2 vs the 2e-2 gate.
  * Biases: 16*(bl-bh) and 16*bg enter PSUM via K=1 ones-row matmuls on the
    PE (so diff = ml/16 - mh = zl - zh exactly); bh rides the host-prebiased
    residual buffer xres = x + bh.
  * Single bf16 x input buffer serves both the spectral lhsT reads and the
    residual adds (no fp32 xf duplicate); group-batched DMAs (3 per group).
  * Epilogue in bf16 (2x DVE modes where SBUF-only): DVE does the two
    PSUM-exit stt/tt ops + bn_stats, Act does sigmoid/final-norm/fp8 pack
    copies, GpSimd does the three SBUF-only adds/muls.
  * rstd via magic-seed + 2 Newton iterations on DVE (avoids Act table swap).
  * Output written bf16, upcast + (host) LN affine applied on the host.
"""

import numpy as np

import concourse.bass as bass
import concourse.tile as tile
from concourse import bacc, mybir
from concourse.bass_utils import run_bass_kernel_spmd

# Problem constants (hardcoded per harness contract)
B = 512
S = 96
D = 512
F = S // 2 + 1  # 49
CUTOFF = F // 3  # 16
EPS = 1e-5
N_CORES = 8
B_LOC = B // N_CORES  # 64
GRP = 8  # samples per packing group
ROWS_PER_GRP = GRP * S  # 768
ROW_TILES = ROWS_PER_GRP // 128  # 6
N_GRPS = B_LOC // GRP  # 8
ROWS_LOC = B_LOC * S  # 6144
SPEC_FREE = 2 * S  # 192

PACK_L_FP8 = True  # xl stream in fp8-e4m3 DoubleRow (else bf16)
WSC = 16.0  # fp8 weight pre-scale (exact power of two)

F32 = mybir.dt.float32
BF16 = mybir.dt.bfloat16
FP8 = mybir.dt.float8e4
DR = mybir.MatmulPerfMode.DoubleRow


def _host_prep(freq_weights, Wl, bl, Wh, bh, Wg, bg):
    """Build spectral matrices, weight layouts and bias rows on the host."""
    import ml_dtypes

    bf16 = ml_dtypes.bfloat16
    f8 = ml_dtypes.float8_e4m3fn

    fw = np.asarray(freq_weights, np.float64)
    eye = np.eye(S)
    RF = np.fft.rfft(eye, axis=0)  # [F, S]
    k = np.arange(F)
    wlo = fw * (k < CUTOFF)
    whi = fw * (k >= CUTOFF)
    A_low = np.fft.irfft(wlo[:, None] * RF, n=S, axis=0)  # [S, S]; xl = A_low @ x_b
    A_high = np.fft.irfft(whi[:, None] * RF, n=S, axis=0)
    AB = np.zeros((128, SPEC_FREE), bf16)
    AB[0:S, 0:S] = A_low.T.astype(bf16)
    AB[0:S, S : 2 * S] = A_high.T.astype(bf16)

    # Gate input is [zl, zh] (post-linear, post-bias), fold host-side:
    #   zg = xl (Wl Wg_top) + xh (Wh Wg_bot) + (bl Wg_top + bh Wg_bot + bg)
    Wl64 = np.asarray(Wl, np.float64)
    Wh64 = np.asarray(Wh, np.float64)
    Wg64 = np.asarray(Wg, np.float64)
    bl64 = np.asarray(bl, np.float64)
    bh64 = np.asarray(bh, np.float64)
    Wgl = Wl64 @ Wg64[:D]
    Wgh = Wh64 @ Wg64[D:]
    bgp = bl64 @ Wg64[:D] + bh64 @ Wg64[D:] + np.asarray(bg, np.float64)

    def chunk4(w, dt):
        return np.ascontiguousarray(
            w.astype(dt).reshape(4, 128, D).transpose(1, 0, 2)
        )  # [128, 4, D], chunk j = w[128j:128j+128, :]

    def pair2(w):  # DoubleRow layout [p, c, i, e] = w[(2c+i)*128+p, e]
        return np.ascontiguousarray(
            (w * WSC).astype(f8).reshape(2, 2, 128, D).transpose(2, 0, 1, 3)
        )

    out = {
        "ab": AB,
        "wh": chunk4(Wh64, bf16),
    }
    if PACK_L_FP8:
        out["wl"] = pair2(Wl64)
        out["wgl"] = pair2(Wgl)
        out["wgh"] = chunk4(Wgh * WSC, bf16)  # x16 exact in bf16: zg PSUM at 16x
        out["blr"] = (bl64 * WSC).astype(bf16).reshape(1, D)
        out["bgr"] = (bgp * WSC).astype(bf16).reshape(1, D)
    else:
        out["wl"] = chunk4(Wl64, bf16)
        out["wgl"] = chunk4(Wgl, bf16)
        out["wgh"] = chunk4(Wgh, bf16)
        out["blr"] = bl64.astype(bf16).reshape(1, D)
        out["bgr"] = bgp.astype(bf16).reshape(1, D)
    # ml ones-row carries 16*(bl - bh): diff = ml/16 - mh = zl - zh exactly,
    # while bh rides the host-prebiased residual input (xres = x + bh).
    sc = WSC if PACK_L_FP8 else 1.0
    out["blr"] = ((bl64 - bh64) * sc).astype(bf16).reshape(1, D)
    out["bh64"] = bh64  # host-side, folded into xres by _in_maps
    out["ones"] = np.ones((1, 128), bf16)
    return out


def _build_nc(use_affine: bool = False):
    del use_affine  # LN affine applied host-side
    nc = bacc.Bacc("TRN2", target_bir_lowering=False, debug=False)

    x_d = nc.declare_dram_parameter("x", [ROWS_LOC + 32, D], BF16, isOutput=False)
    xres_d = nc.declare_dram_parameter("xres", [ROWS_LOC, D], BF16, isOutput=False)
    ab_d = nc.declare_dram_parameter("ab", [128, SPEC_FREE], BF16, isOutput=False)
    if PACK_L_FP8:
        wl_d = nc.declare_dram_parameter("wl", [128, 2, 2, D], FP8, isOutput=False)
        wgl_d = nc.declare_dram_parameter("wgl", [128, 2, 2, D], FP8, isOutput=False)
    else:
        wl_d = nc.declare_dram_parameter("wl", [128, 4, D], BF16, isOutput=False)
        wgl_d = nc.declare_dram_parameter("wgl", [128, 4, D], BF16, isOutput=False)
    wh_d = nc.declare_dram_parameter("wh", [128, 4, D], BF16, isOutput=False)
    wgh_d = nc.declare_dram_parameter("wgh", [128, 4, D], BF16, isOutput=False)
    blr_d = nc.declare_dram_parameter("blr", [1, D], BF16, isOutput=False)
    bgr_d = nc.declare_dram_parameter("bgr", [1, D], BF16, isOutput=False)
    ones_d = nc.declare_dram_parameter("ones", [1, 128], BF16, isOutput=False)
    out_d = nc.declare_dram_parameter("out", [ROWS_LOC, D], BF16, isOutput=True)

    from contextlib import ExitStack

    with tile.TileContext(nc) as tc, ExitStack() as ctx:
        singles = ctx.enter_context(tc.tile_pool(name="singles", bufs=1))
        xs_pool = ctx.enter_context(tc.tile_pool(name="xs", bufs=3))
        xr_pool = ctx.enter_context(tc.tile_pool(name="xr", bufs=3))
        pl_pool = ctx.enter_context(tc.tile_pool(name="packl", bufs=3))
        ph_pool = ctx.enter_context(tc.tile_pool(name="packh", bufs=3))
        eplg = ctx.enter_context(tc.tile_pool(name="eplg", bufs=12))
        ypool = ctx.enter_context(tc.tile_pool(name="ypool", bufs=14))
        ybig_pool = ctx.enter_context(tc.tile_pool(name="ybig", bufs=3))
        small = ctx.enter_context(tc.tile_pool(name="small", bufs=6))
        nwt = ctx.enter_context(tc.tile_pool(name="nwt", bufs=8))
        spec_psum = ctx.enter_context(tc.tile_pool(name="specp", bufs=2, space="PSUM"))
        ml_psum = ctx.enter_context(tc.tile_pool(name="mlp", bufs=2, space="PSUM"))
        mh_psum = ctx.enter_context(tc.tile_pool(name="mhp", bufs=2, space="PSUM"))
        zg_psum = ctx.enter_context(tc.tile_pool(name="zgp", bufs=2, space="PSUM"))

        # --- constants ---
        ab_sb = singles.tile([128, SPEC_FREE], BF16)
        nc.sync.dma_start(out=ab_sb[:], in_=ab_d[:])
        if PACK_L_FP8:
            wl_sb = singles.tile([128, 2, 2, D], FP8)
            wgl_sb = singles.tile([128, 2, 2, D], FP8)
        else:
            wl_sb = singles.tile([128, 4, D], BF16)
            wgl_sb = singles.tile([128, 4, D], BF16)
        wh_sb = singles.tile([128, 4, D], BF16)
        wgh_sb = singles.tile([128, 4, D], BF16)
        blr_sb = singles.tile([1, D], BF16)
        bgr_sb = singles.tile([1, D], BF16)
        ones_sb = singles.tile([1, 128], BF16)

        def load_weights():
            nc.sync.dma_start(out=wl_sb[:], in_=wl_d[:])
            nc.sync.dma_start(out=wgl_sb[:], in_=wgl_d[:])
            nc.sync.dma_start(out=wh_sb[:], in_=wh_d[:])
            nc.sync.dma_start(out=wgh_sb[:], in_=wgh_d[:])
            nc.sync.dma_start(out=blr_sb[:], in_=blr_d[:])
            nc.sync.dma_start(out=bgr_sb[:], in_=bgr_d[:])
            nc.sync.dma_start(out=ones_sb[:], in_=ones_d[:])

        # quake magic, adjusted to take bits of (v+eps)/2 (-0x400000) and to
        # emit the NEGATED seed via sign-bit fold (+0x80000000): 0xDEF759DF
        magic_sb = singles.tile([128, ROW_TILES], mybir.dt.int32)
        nc.vector.memset(magic_sb[:], 0xDEF759DF - (1 << 32))

        def mm(out_ap, lhsT, rhs, start, stop, perf_mode=None):
            nc.tensor.matmul(
                out_ap, lhsT=lhsT, rhs=rhs, start=start, stop=stop, perf_mode=perf_mode
            )

        xs_tiles = {}
        xr_tiles = {}

        def load_group(g):
            if g >= N_GRPS or g in xs_tiles:
                return
            xs = xs_pool.tile([128, GRP, D], BF16, tag="xs", name=f"xs_{g}")
            nc.sync.dma_start(
                out=xs[:],
                in_=bass.AP(
                    tensor=x_d,
                    offset=g * GRP * S * D,
                    ap=[[D, 128], [S * D, GRP], [1, D]],
                ),
            )
            xs_tiles[g] = xs
            xr = xr_pool.tile([128, ROW_TILES, D], BF16, tag="xr", name=f"xr_{g}")
            nc.sync.dma_start(
                out=xr[:],
                in_=bass.AP(
                    tensor=xres_d,
                    offset=g * ROWS_PER_GRP * D,
                    ap=[[D, 128], [128 * D, ROW_TILES], [1, D]],
                ),
            )
            xr_tiles[g] = xr

        packs = {}

        def emit_spectral(gi, i):
            if gi >= N_GRPS:
                return
            if i == 0:
                pdt = FP8 if PACK_L_FP8 else BF16
                packs[gi] = (
                    pl_pool.tile([128, 2, 2, GRP, S], pdt, tag="pl", name=f"pl_{gi}"),
                    ph_pool.tile([128, 4, GRP, S], BF16, tag="ph", name=f"ph_{gi}"),
                )
            pl, ph = packs[gi]
            xs = xs_tiles[gi]
            for t in range(2):
                spec_ps = spec_psum.tile([128, 2, SPEC_FREE], F32, tag="spec")
                for cc in range(2):
                    j = 2 * t + cc
                    mm(
                        spec_ps[:, cc, :],
                        xs[:, i, 128 * j : 128 * (j + 1)],
                        ab_sb[:],
                        start=True,
                        stop=True,
                    )
                # xl^T -> pack_L (fp8, Act); xh^T -> pack_H (bf16, DVE/Act split)
                nc.scalar.copy(out=pl[:, t, :, i, :], in_=spec_ps[:, :, 0:S])
                if t == 0:
                    nc.vector.tensor_copy(
                        out=ph[:, 0:2, i, :], in_=spec_ps[:, :, S : 2 * S]
                    )
                else:
                    nc.scalar.copy(
                        out=ph[:, 2:4, i, :], in_=spec_ps[:, :, S : 2 * S]
                    )

        # prologue: stage group-0 x in two halves so spectral starts early,
        # then channel weights (needed a few us later), then the residual rows
        xs0 = xs_pool.tile([128, GRP, D], BF16, tag="xs", name="xs_0")
        for h in range(2):
            nc.sync.dma_start(
                out=xs0[:, 4 * h : 4 * h + 4, :],
                in_=bass.AP(
                    tensor=x_d,
                    offset=4 * h * S * D,
                    ap=[[D, 128], [S * D, 4], [1, D]],
                ),
            )
        xs_tiles[0] = xs0
        load_weights()
        xr0 = xr_pool.tile([128, ROW_TILES, D], BF16, tag="xr", name="xr_0")
        nc.sync.dma_start(
            out=xr0[:],
            in_=bass.AP(
                tensor=xres_d, offset=0, ap=[[D, 128], [128 * D, ROW_TILES], [1, D]]
            ),
        )
        xr_tiles[0] = xr0
        for i in range(GRP):
            emit_spectral(0, i)

        def ln_tail(grp, half, mvg, y_tiles):
            # rstd = 1/sqrt(var+eps): magic seed + 1 Newton step, 7 DVE ops.
            # hv = (var+eps)/2; seed taken from hv's bits with the magic
            # constant pre-shifted by 0x400000 (exact exponent identity), and
            # carried NEGATED so the Newton step fuses into one stt:
            #   yk' = (bits(hv)>>1) - magic2 = -y0
            #   rstd = (hv*yk'^2 - 1.5) * yk' = y0*(1.5 - hv*y0^2)
            R = ROW_TILES // 2
            cs = slice(half * R, (half + 1) * R)
            hv = nwt.tile([128, R], F32, tag="hv")
            nc.vector.tensor_scalar(
                out=hv[:], in0=mvg[:, 1, cs], scalar1=EPS, scalar2=0.5,
                op0=mybir.AluOpType.add, op1=mybir.AluOpType.mult,
            )
            ti = nwt.tile([128, R], mybir.dt.int32, tag="ti")
            nc.vector.tensor_scalar(
                out=ti[:], in0=hv[:].bitcast(mybir.dt.int32), scalar1=1,
                scalar2=None, op0=mybir.AluOpType.arith_shift_right,
            )
            yk = nwt.tile([128, R], F32, tag="yk")
            nc.vector.tensor_sub(
                yk[:].bitcast(mybir.dt.int32), magic_sb[:, 0:R], ti[:]
            )
            aa = nwt.tile([128, R], F32, tag="aa")
            nc.vector.tensor_mul(aa[:], yk[:], yk[:])
            cc = nwt.tile([128, R], F32, tag="cc")
            nc.vector.tensor_mul(cc[:], aa[:], hv[:])
            rstd3 = nwt.tile([128, R], F32, tag="y2")
            nc.vector.scalar_tensor_tensor(
                out=rstd3[:], in0=cc[:], scalar=1.5, in1=yk[:],
                op0=mybir.AluOpType.subtract, op1=mybir.AluOpType.mult,
            )
            nmb3 = nwt.tile([128, R], F32, tag="nmb3")
            nc.vector.scalar_tensor_tensor(
                out=nmb3[:], in0=mvg[:, 0, cs], scalar=-1.0, in1=rstd3[:],
                op0=mybir.AluOpType.mult, op1=mybir.AluOpType.mult,
            )

            ybig = ybig_pool.tile([128, R, D], BF16, tag="ybig")
            for r in range(R):
                nc.scalar.activation(
                    ybig[:, r, :],
                    y_tiles[half * R + r][:],
                    mybir.ActivationFunctionType.Identity,
                    bias=nmb3[:, r : r + 1],
                    scale=rstd3[:, r : r + 1],
                )
            nc.sync.dma_start(
                out=bass.AP(
                    tensor=out_d,
                    offset=(grp * ROWS_PER_GRP + half * R * 128) * D,
                    ap=[[D, 128], [128 * D, R], [1, D]],
                ),
                in_=ybig[:],
            )

        inv = 1.0 / WSC if PACK_L_FP8 else 1.0

        for grp in range(N_GRPS):
            load_group(grp + 1)
            pl, ph = packs[grp]
            packL = pl.rearrange("p c i m s -> p c i (m s)")
            packH = ph.rearrange("p j m s -> p j (m s)")
            xr = xr_tiles[grp]

            mvg = small.tile([128, 2, ROW_TILES], F32, tag="mvg")
            y_tiles = []
            for r in range(ROW_TILES):
                rs = slice(128 * r, 128 * (r + 1))

                ml_ps = ml_psum.tile([128, D], F32, tag="ml", name=f"ml_{grp}_{r}")
                mh_ps = mh_psum.tile([128, D], F32, tag="mh", name=f"mh_{grp}_{r}")
                zg_ps = zg_psum.tile([128, D], F32, tag="zg", name=f"zg_{grp}_{r}")

                if PACK_L_FP8:
                    # DR matmuls interleaved with bf16 ones: DoubleRow
                    # LDWEIGHTS (no FWL) hides behind the bf16 streams
                    mm(ml_ps[:], packL[:, 0, :, rs], wl_sb[:, 0], True, False,
                       perf_mode=DR)
                    mm(mh_ps[:], packH[:, 0, rs], wh_sb[:, 0], True, False)
                    mm(zg_ps[:], packL[:, 0, :, rs], wgl_sb[:, 0], True, False,
                       perf_mode=DR)
                    mm(mh_ps[:], packH[:, 1, rs], wh_sb[:, 1], False, False)
                    mm(ml_ps[:], packL[:, 1, :, rs], wl_sb[:, 1], False, False,
                       perf_mode=DR)
                    mm(mh_ps[:], packH[:, 2, rs], wh_sb[:, 2], False, False)
                    mm(zg_ps[:], packL[:, 1, :, rs], wgl_sb[:, 1], False, False,
                       perf_mode=DR)
                    mm(mh_ps[:], packH[:, 3, rs], wh_sb[:, 3], False, True)
                    for j in range(4):
                        mm(zg_ps[:], packH[:, j, rs], wgh_sb[:, j], False, False)
                else:
                    for c in range(4):
                        mm(ml_ps[:], packL[:, c, rs], wl_sb[:, c], c == 0, False)
                        mm(zg_ps[:], packL[:, c, rs], wgl_sb[:, c], c == 0, False)
                    for j in range(4):
                        mm(mh_ps[:], packH[:, j, rs], wh_sb[:, j], j == 0, j == 3)
                        mm(zg_ps[:], packH[:, j, rs], wgh_sb[:, j], False, False)
                # K=1 ones-row bias matmuls: + bl (x16) and + bg (x16)
                mm(ml_ps[:], ones_sb[0:1, :], blr_sb[0:1, :], False, True)
                mm(zg_ps[:], ones_sb[0:1, :], bgr_sb[0:1, :], False, True)

                # epilogue
                g_sb = eplg.tile([128, D], BF16, tag="g")
                nc.scalar.activation(
                    g_sb[:], zg_ps[:], mybir.ActivationFunctionType.Sigmoid,
                    scale=inv,
                )
                # single PSUM exit for mh: cheap cast, then SBUF-only consumers
                mhq = eplg.tile([128, D], BF16, tag="mhq")
                nc.vector.tensor_copy(out=mhq[:], in_=mh_ps[:])
                # diff = ml/16 - mh = zl - zh (bl-bh folded into ml's ones-row)
                diff = eplg.tile([128, D], BF16, tag="diff")
                nc.vector.scalar_tensor_tensor(
                    out=diff[:], in0=ml_ps[:], scalar=inv, in1=mhq[:],
                    op0=mybir.AluOpType.mult, op1=mybir.AluOpType.subtract,
                )
                # s = mh + (x + bh) = zh + x  (bh pre-added into xres on host)
                s_sb = eplg.tile([128, D], BF16, tag="s")
                nc.vector.tensor_add(s_sb[:], mhq[:], xr[:, r, :])
                p_sb = eplg.tile([128, D], BF16, tag="p")
                nc.gpsimd.tensor_mul(p_sb[:], g_sb[:], diff[:])
                y_sb = ypool.tile([128, D], BF16, tag="y")
                y_tiles.append(y_sb)
                nc.gpsimd.tensor_add(y_sb[:], s_sb[:], p_sb[:])

                stats = small.tile([128, 6], F32, tag="stats")
                nc.vector.bn_stats(out=stats[:], in_=y_sb[:])
                nc.vector.bn_aggr(out=mvg[:, :, r], in_=stats[:])

                emit_spectral(grp + 1, r)
                if r == ROW_TILES // 2 - 1:
                    ln_tail(grp, 0, mvg, y_tiles)

            for i in range(ROW_TILES, GRP):
                emit_spectral(grp + 1, i)
            del packs[grp], xs_tiles[grp]
            ln_tail(grp, 1, mvg, y_tiles)
            del xr_tiles[grp]

    nc.finalize()
    return nc


_NC_CACHE = {}


def _in_maps(params, x):
    import ml_dtypes

    bf16 = ml_dtypes.bfloat16
    maps = []
    bh64 = params["bh64"]
    shared = {k: v for k, v in params.items() if k != "bh64"}
    for i in range(N_CORES):
        m = dict(shared)
        xc = np.asarray(x[i * B_LOC : (i + 1) * B_LOC], np.float32).reshape(
            ROWS_LOC, D
        )
        xpad = np.zeros((ROWS_LOC + 32, D), bf16)
        xpad[:ROWS_LOC] = xc.astype(bf16)
        m["x"] = xpad
        m["xres"] = (xc + bh64.astype(np.float32)).astype(bf16)
        maps.append(m)
    return maps


def kernel(x, freq_weights, Wl, bl, Wh, bh, Wg, bg, gamma, beta):
    x = np.asarray(x, np.float32)
    gamma = np.asarray(gamma, np.float32)
    beta = np.asarray(beta, np.float32)

    params = _host_prep(freq_weights, Wl, bl, Wh, bh, Wg, bg)

    if "nc" not in _NC_CACHE:
        _NC_CACHE["nc"] = _build_nc()
    nc = _NC_CACHE["nc"]

    core_ids = list(range(N_CORES))
    in_maps = _in_maps(params, x)

    res = None
    last_err = None
    for _attempt in range(3):
        try:
            res = run_bass_kernel_spmd(nc, in_maps, core_ids)
            break
        except Exception as e:  # transient NRT device errors on fresh NEFFs
            last_err = e
    if res is None:
        raise last_err
    out = np.concatenate(
        [
            np.asarray(res.results[i]["out"])
            .astype(np.float32)
            .reshape(B_LOC, S, D)
            for i in range(N_CORES)
        ],
        axis=0,
    )
    if not (np.all(gamma == 1.0) and np.all(beta == 0.0)):
        out = out * gamma.reshape(1, 1, D) + beta.reshape(1, 1, D)
    return out
